# revision 9
# baseline (speedup 1.0000x reference)
"""Contrastive-loss kernel for 8 trn2 NeuronCores (Bass/Tile, SPMD).

Sharding: pixel-ownership data parallel. Core k owns pixels
[k*32768, (k+1)*32768) of the flattened (B*H*W) pixel space and receives the
C-major feature slice feat[b, :, hw] for its range ([128, 32768] f32 —
contiguous in the NCHW input). Anchor/pos/neg index entries are routed to the
owning core as int16 local indices. Each core:

  P0  streams its slice, PE-transposes 128x128 chunks, and materializes an
      SBUF-resident bf16 row table (token i -> partition i%128, 256B stripe
      i//128 — the layout dma_gather's SBUF-source mode expects).
  P1  dma_gathers its anchor/neg/pos rows into [C, n] bf16 operand tiles and
      reduces its pos partial sum (f32, exact pad correction).
  P2  one AllGather ships the neg columns + bitcast pos partial; compaction
      DMAs drop pad columns; partials sum to pos_mean.
  P3  per 128-anchor tile: bf16 matmuls (K=C=128 on partitions) into PSUM,
      ACT Exp(scale=1/T) with fused accum_out row-sums, then
      loss = exp_pos / (exp_pos + sum_negs exp).

The loss for padded anchors is garbage and dropped on the host, which also
inverse-permutes per-core results back to the original anchor order.
"""

import math

import numpy as np

TEMP = 0.1
B, C, H, W = 4, 128, 256, 256
N_PIX = B * H * W
N_CORES = 8
RANGE = N_PIX // N_CORES  # 32768 pixels per core
ASSEMBLY_CHUNK = 2048  # PSUM cols drained per ACT exp instruction (4 banks)
MM_N = 512  # matmul moving-operand free dim


def _ceil_to(x: int, m: int) -> int:
    return ((x + m - 1) // m) * m


def _wrap_idx16(vals: np.ndarray, num_idxs: int) -> np.ndarray:
    """int16 index layout for dma_gather: [128, num_idxs//16], index i at
    [i%16, i//16], replicated across the 8 gpsimd cores' partition blocks."""
    assert num_idxs % 16 == 0
    arr = np.zeros((16, num_idxs // 16), dtype=np.int16)
    flat = np.zeros(num_idxs, dtype=np.int16)
    flat[: len(vals)] = vals.astype(np.int16)
    arr[:, :] = flat.reshape(num_idxs // 16, 16).T
    return np.tile(arr, (8, 1))


def _build_program(NA: int, NN: int, NP: int, nn_list: list[int], n_pos_total: int):
    import concourse.bacc as bacc
    import concourse.mybir as mybir
    import concourse.tile as tile
    from concourse.masks import make_identity

    f32 = mybir.dt.float32
    bf16 = mybir.dt.bfloat16
    i16 = mybir.dt.int16

    NN_TOT = sum(nn_list)
    n_atile = NA // 128

    nc = bacc.Bacc(num_devices=N_CORES)

    feat_in = nc.dram_tensor("feat", [C, RANGE], f32, kind="ExternalInput")
    a_idx_in = nc.dram_tensor("a_idx", [128, NA // 16], i16, kind="ExternalInput")
    n_idx_in = nc.dram_tensor("n_idx", [128, NN // 16], i16, kind="ExternalInput")
    p_idx_in = nc.dram_tensor("p_idx", [128, NP // 16], i16, kind="ExternalInput")
    # number of padded pos entries for this core, replicated [128, 1] f32
    p_pad_in = nc.dram_tensor("p_pad", [128, 1], f32, kind="ExternalInput")
    loss_out = nc.dram_tensor("loss", [128, n_atile], f32, kind="ExternalOutput")

    cc_in = nc.dram_tensor("cc_in", [C, NN + 2], bf16)
    cc_out = nc.dram_tensor("cc_out", [N_CORES, C, NN + 2], bf16, addr_space="Shared")

    with tile.TileContext(nc) as tc:
        with (
            tc.tile_pool(name="const", bufs=1) as const_pool,
            tc.tile_pool(name="big", bufs=1) as big,
            tc.tile_pool(name="stat", bufs=1) as stat,
        ):
            identity = const_pool.tile([128, 128], f32)
            make_identity(nc, identity[:])

            table = big.tile([128, RANGE], bf16)

            # ---- P0: build the bf16 row table from the f32 C-major slice ----
            with (
                tc.tile_pool(name="p0io", bufs=4) as p0io,
                tc.tile_pool(name="p0ps", bufs=4, space="PSUM") as p0ps,
            ):
                for t in range(RANGE // 512):
                    ft = p0io.tile([128, 512], f32)
                    nc.sync.dma_start(ft[:], feat_in[:, t * 512 : (t + 1) * 512])
                    ps = p0ps.tile([128, 512], f32)
                    for j in range(4):
                        nc.tensor.transpose(
                            ps[:, j * 128 : (j + 1) * 128],
                            ft[:, j * 128 : (j + 1) * 128],
                            identity[:],
                        )
                    nc.scalar.copy(table[:, t * 512 : (t + 1) * 512], ps[:])

            # ---- P1: gathers ----
            Ag = big.tile([128, 1, NA], bf16)
            NgOwn = big.tile([128, 1, NN], bf16)

            GCH = 512  # max indices per dma_gather call (device limit)

            def gather_chunked(out_tile, idx_tile, n_total):
                for o in range(0, n_total, GCH):
                    w = min(GCH, n_total - o)
                    nc.gpsimd.dma_gather(
                        out_tile[:, :, o : o + w], table[:],
                        idx_tile[:, o // 16 : (o + w) // 16],
                        num_idxs=w, num_idxs_reg=w, elem_size=128,
                        transpose=True,
                        sbuf_tokens_per_rank=128, sbuf_free_dim_per_rank=256,
                    )

            with tc.tile_pool(name="p1", bufs=1) as p1:
                a_idx = p1.tile([128, NA // 16], i16)
                nc.sync.dma_start(a_idx[:], a_idx_in[:])
                gather_chunked(Ag, a_idx, NA)
                n_idx = p1.tile([128, NN // 16], i16)
                nc.sync.dma_start(n_idx[:], n_idx_in[:])
                gather_chunked(NgOwn, n_idx, NN)
                p_idx = p1.tile([128, NP // 16], i16)
                nc.sync.dma_start(p_idx[:], p_idx_in[:])
                Pg = p1.tile([128, 1, NP], bf16)
                gather_chunked(Pg, p_idx, NP)

                # pos partial sum with exact pad correction: pads all point at
                # local row 0, which is also what column NP-1 holds when any
                # pad exists; when there are no pads p_pad is 0.
                p_pad = p1.tile([128, 1], f32)
                nc.sync.dma_start(p_pad[:], p_pad_in[:])
                pos_red = stat.tile([128, 1], f32)
                nc.vector.tensor_reduce(
                    pos_red[:], Pg[:], axis=mybir.AxisListType.XY,
                    op=mybir.AluOpType.add,
                )
                corr = p1.tile([128, 1], f32)
                nc.vector.tensor_scalar_mul(corr[:], Pg[:, 0, NP - 1 : NP], p_pad[:])
                partial = stat.tile([128, 1], f32)
                nc.vector.tensor_sub(partial[:], pos_red[:], corr[:])
                # ship the f32 partial as an exact hi+lo bf16 pair
                p_hilo = stat.tile([128, 2], bf16)
                nc.vector.tensor_copy(p_hilo[:, 0:1], partial[:])
                nc.vector.tensor_sub(p_hilo[:, 1:2], partial[:], p_hilo[:, 0:1])

                # ---- P2: exchange ----
                nc.sync.dma_start(cc_in[:, 0:NN], NgOwn[:, 0, :])
                nc.sync.dma_start(cc_in[:, NN : NN + 2], p_hilo[:])
            nc.gpsimd.collective_compute(
                "AllGather",
                mybir.AluOpType.bypass,
                replica_groups=[list(range(N_CORES))],
                ins=[cc_in[:]],
                outs=[cc_out[:]],
            )

            # compact valid neg columns (drop pads) straight out of DRAM
            NgD = big.tile([128, NN_TOT], bf16)
            off = 0
            for r in range(N_CORES):
                w = nn_list[r]
                if w == 0:
                    continue
                nc.sync.dma_start(
                    NgD[:, off : off + w],
                    cc_out[r, :, 0:w],
                )
                off += w

            # pos_mean from the 8 bitcast partials
            with tc.tile_pool(name="p2", bufs=1) as p2:
                ps8 = p2.tile([128, N_CORES, 2], bf16)
                nc.sync.dma_start(
                    ps8[:],
                    cc_out[:, :, NN : NN + 2].rearrange("r c j -> c r j"),
                )
                pos_sum = p2.tile([128, 1], f32)
                nc.vector.tensor_reduce(
                    pos_sum[:], ps8[:], axis=mybir.AxisListType.XY,
                    op=mybir.AluOpType.add,
                )
                pos_mean = stat.tile([128, 1], bf16)
                nc.scalar.mul(pos_mean[:], pos_sum[:], 1.0 / float(n_pos_total))

            # ---- P3: matmul + exp + rowsum ----
            exp_pos = stat.tile([128, n_atile], f32)
            with tc.tile_pool(name="pps", bufs=1, space="PSUM") as pps:
                zpos = pps.tile([128, n_atile], f32)
                for a in range(n_atile):
                    nc.tensor.matmul(
                        zpos[:, a : a + 1],
                        Ag[:, 0, a * 128 : (a + 1) * 128],
                        pos_mean[:],
                    )
                nc.scalar.activation(
                    exp_pos[:], zpos[:], mybir.ActivationFunctionType.Exp,
                    scale=1.0 / TEMP,
                )

            chunks = []
            o = 0
            while o < NN_TOT:
                w = min(ASSEMBLY_CHUNK, NN_TOT - o)
                chunks.append((o, w))
                o += w

            loss_sb = stat.tile([128, n_atile], f32)
            with (
                tc.tile_pool(name="mmps", bufs=2, space="PSUM") as mmps,
                tc.tile_pool(name="scrap", bufs=3) as scrap_pool,
                tc.tile_pool(name="acc", bufs=2) as acc_pool,
            ):
                for a in range(n_atile):
                    lhsT = Ag[:, 0, a * 128 : (a + 1) * 128]
                    sums = acc_pool.tile([128, len(chunks)], f32)
                    for ci, (o, w) in enumerate(chunks):
                        zp = mmps.tile([128, ASSEMBLY_CHUNK], f32)
                        for s in range(0, w, MM_N):
                            n = min(MM_N, w - s)
                            nc.tensor.matmul(
                                zp[:, s : s + n],
                                lhsT,
                                NgD[:, o + s : o + s + n],
                            )
                        scrap = scrap_pool.tile([128, ASSEMBLY_CHUNK], bf16)
                        nc.scalar.activation(
                            scrap[:, 0:w], zp[:, 0:w],
                            mybir.ActivationFunctionType.Exp,
                            scale=1.0 / TEMP,
                            accum_out=sums[:, ci : ci + 1],
                        )
                    S = acc_pool.tile([128, 1], f32)
                    nc.vector.tensor_reduce(
                        S[:], sums[:], axis=mybir.AxisListType.X,
                        op=mybir.AluOpType.add,
                    )
                    denom = acc_pool.tile([128, 1], f32)
                    nc.vector.tensor_add(denom[:], S[:], exp_pos[:, a : a + 1])
                    rec = acc_pool.tile([128, 1], f32)
                    nc.vector.reciprocal(rec[:], denom[:])
                    nc.vector.tensor_mul(
                        loss_sb[:, a : a + 1], rec[:], exp_pos[:, a : a + 1]
                    )

            nc.sync.dma_start(loss_out[:], loss_sb[:])

    nc.finalize()
    return nc




def _route(anchor_idx, pos_idx, neg_idx):
    """Route index entries to the owning core as local indices."""
    anchor_idx = np.asarray(anchor_idx).astype(np.int64)
    pos_idx = np.asarray(pos_idx).astype(np.int64)
    neg_idx = np.asarray(neg_idx).astype(np.int64)

    a_owner, a_local = anchor_idx // RANGE, anchor_idx % RANGE
    p_owner, p_local = pos_idx // RANGE, pos_idx % RANGE
    n_owner, n_local = neg_idx // RANGE, neg_idx % RANGE

    a_lists, p_lists, n_lists, perms = [], [], [], []
    for k in range(N_CORES):
        sel = np.nonzero(a_owner == k)[0]
        perms.append(sel)
        a_lists.append(a_local[sel])
        p_lists.append(p_local[p_owner == k])
        n_lists.append(n_local[n_owner == k])

    NA = max(128, _ceil_to(max(len(x) for x in a_lists), 128))
    NP = max(128, _ceil_to(max(len(x) for x in p_lists), 128))
    NN = max(128, _ceil_to(max(len(x) for x in n_lists), 128))
    nn_list = [len(x) for x in n_lists]
    return a_lists, p_lists, n_lists, perms, NA, NP, NN, nn_list


def make_in_maps(fslices, a_lists, p_lists, n_lists, NA, NP, NN):
    in_maps = []
    for k in range(N_CORES):
        in_maps.append(
            {
                "feat": fslices[k],
                "a_idx": _wrap_idx16(a_lists[k], NA),
                "n_idx": _wrap_idx16(n_lists[k], NN),
                "p_idx": _wrap_idx16(p_lists[k], NP),
                "p_pad": np.full((128, 1), NP - len(p_lists[k]), dtype=np.float32),
            }
        )
    return in_maps


def _prepare(feat, anchor_idx, pos_idx, neg_idx):
    """Host-side sharding: slice feat per core, route index entries to the
    owning core as local int16 indices, pad to common shapes."""
    feat = np.ascontiguousarray(np.asarray(feat), dtype=np.float32)
    fview = feat.reshape(B, C, (H * W))

    a_lists, p_lists, n_lists, perms, NA, NP, NN, nn_list = _route(
        anchor_idx, pos_idx, neg_idx
    )

    per_image = H * W // RANGE  # core ranges per image
    fslices = []
    for k in range(N_CORES):
        b, part = k // per_image, k % per_image
        fslices.append(
            np.ascontiguousarray(fview[b, :, part * RANGE : (part + 1) * RANGE])
        )
    in_maps = make_in_maps(fslices, a_lists, p_lists, n_lists, NA, NP, NN)
    return in_maps, perms, a_lists, NA, NP, NN, nn_list


def run_sharded(feat, anchor_idx, pos_idx, neg_idx, trace=False):
    from concourse.bass_utils import run_bass_kernel_spmd

    in_maps, perms, a_lists, NA, NP, NN, nn_list = _prepare(
        feat, anchor_idx, pos_idx, neg_idx
    )
    nc = _build_program(NA, NN, NP, nn_list, int(np.asarray(pos_idx).size))
    res = run_bass_kernel_spmd(
        nc, in_maps, core_ids=list(range(N_CORES)), trace=trace
    )
    n_anchor = sum(len(p) for p in perms)
    loss = np.zeros(n_anchor, dtype=np.float32)
    for k in range(N_CORES):
        buf = res.results[k]["loss"]  # [128, NA//128]
        vals = buf.T.reshape(-1)[: len(perms[k])]
        loss[perms[k]] = vals
    return loss, res


def kernel(feat, anchor_idx, pos_idx, neg_idx):
    out_dtype = np.asarray(feat).dtype
    loss, _ = run_sharded(feat, anchor_idx, pos_idx, neg_idx, trace=False)
    return loss.astype(out_dtype)


# revision 10
# speedup vs baseline: 1.0313x; 1.0313x over previous
"""Contrastive-loss kernel for 8 trn2 NeuronCores (Bass/Tile, SPMD).

Sharding: pixel-ownership data parallel. Core k owns pixels
[k*32768, (k+1)*32768) of the flattened (B*H*W) pixel space and receives the
C-major feature slice feat[b, :, hw] for its range ([128, 32768] f32 —
contiguous in the NCHW input). Anchor/pos/neg index entries are routed to the
owning core as int16 local indices. Each core:

  P0  streams its slice, PE-transposes 128x128 chunks, and materializes an
      SBUF-resident bf16 row table (token i -> partition i%128, 256B stripe
      i//128 — the layout dma_gather's SBUF-source mode expects).
  P1  dma_gathers its anchor/neg/pos rows into [C, n] bf16 operand tiles and
      reduces its pos partial sum (f32, exact pad correction).
  P2  one AllGather ships the neg columns + bitcast pos partial; compaction
      DMAs drop pad columns; partials sum to pos_mean.
  P3  per 128-anchor tile: bf16 matmuls (K=C=128 on partitions) into PSUM,
      ACT Exp(scale=1/T) with fused accum_out row-sums, then
      loss = exp_pos / (exp_pos + sum_negs exp).

The loss for padded anchors is garbage and dropped on the host, which also
inverse-permutes per-core results back to the original anchor order.
"""

import math

import numpy as np

TEMP = 0.1
B, C, H, W = 4, 128, 256, 256
N_PIX = B * H * W
N_CORES = 8
RANGE = N_PIX // N_CORES  # 32768 pixels per core
ASSEMBLY_CHUNK = 2048  # PSUM cols drained per ACT exp instruction (4 banks)
MM_N = 512  # matmul moving-operand free dim


def _ceil_to(x: int, m: int) -> int:
    return ((x + m - 1) // m) * m


def _wrap_idx16(vals: np.ndarray, num_idxs: int) -> np.ndarray:
    """int16 index layout for dma_gather: [128, num_idxs//16], index i at
    [i%16, i//16], replicated across the 8 gpsimd cores' partition blocks."""
    assert num_idxs % 16 == 0
    arr = np.zeros((16, num_idxs // 16), dtype=np.int16)
    flat = np.zeros(num_idxs, dtype=np.int16)
    flat[: len(vals)] = vals.astype(np.int16)
    arr[:, :] = flat.reshape(num_idxs // 16, 16).T
    return np.tile(arr, (8, 1))


def _build_program(NA: int, NN: int, NP: int, nn_list: list[int], n_pos_total: int):
    import concourse.bacc as bacc
    import concourse.mybir as mybir
    import concourse.tile as tile
    from concourse.masks import make_identity

    f32 = mybir.dt.float32
    bf16 = mybir.dt.bfloat16
    i16 = mybir.dt.int16

    NN_TOT = sum(nn_list)
    n_atile = NA // 128

    nc = bacc.Bacc(num_devices=N_CORES)

    feat_in = nc.dram_tensor("feat", [C, RANGE], f32, kind="ExternalInput")
    a_idx_in = nc.dram_tensor("a_idx", [128, NA // 16], i16, kind="ExternalInput")
    n_idx_in = nc.dram_tensor("n_idx", [128, NN // 16], i16, kind="ExternalInput")
    p_idx_in = nc.dram_tensor("p_idx", [128, NP // 16], i16, kind="ExternalInput")
    # number of padded pos entries for this core, replicated [128, 1] f32
    p_pad_in = nc.dram_tensor("p_pad", [128, 1], f32, kind="ExternalInput")
    loss_out = nc.dram_tensor("loss", [128, n_atile], f32, kind="ExternalOutput")

    cc_in = nc.dram_tensor("cc_in", [C, NN + 2], bf16)
    cc_out = nc.dram_tensor("cc_out", [N_CORES, C, NN + 2], bf16, addr_space="Shared")

    with tile.TileContext(nc) as tc:
        with (
            tc.tile_pool(name="const", bufs=1) as const_pool,
            tc.tile_pool(name="big", bufs=1) as big,
            tc.tile_pool(name="stat", bufs=1) as stat,
        ):
            identity = const_pool.tile([128, 128], f32)
            make_identity(nc, identity[:])

            table = big.tile([128, RANGE], bf16)

            # ---- P0: build the bf16 row table from the f32 C-major slice ----
            with (
                tc.tile_pool(name="p0io", bufs=4) as p0io,
                tc.tile_pool(name="p0ps", bufs=4, space="PSUM") as p0ps,
            ):
                P0T = 2048
                for t in range(RANGE // P0T):
                    ft = p0io.tile([128, P0T], f32)
                    nc.sync.dma_start(ft[:], feat_in[:, t * P0T : (t + 1) * P0T])
                    for h in range(P0T // 512):
                        ps = p0ps.tile([128, 512], f32)
                        for j in range(4):
                            c = h * 512 + j * 128
                            nc.tensor.transpose(
                                ps[:, j * 128 : (j + 1) * 128],
                                ft[:, c : c + 128],
                                identity[:],
                            )
                        nc.scalar.copy(
                            table[:, t * P0T + h * 512 : t * P0T + (h + 1) * 512],
                            ps[:],
                        )

            # ---- P1: gathers ----
            Ag = big.tile([128, 1, NA], bf16)
            NgOwn = big.tile([128, 1, NN], bf16)

            GCH = 512  # max indices per dma_gather call (device limit)

            def gather_chunked(out_tile, idx_tile, n_total):
                for o in range(0, n_total, GCH):
                    w = min(GCH, n_total - o)
                    nc.gpsimd.dma_gather(
                        out_tile[:, :, o : o + w], table[:],
                        idx_tile[:, o // 16 : (o + w) // 16],
                        num_idxs=w, num_idxs_reg=w, elem_size=128,
                        transpose=True,
                        sbuf_tokens_per_rank=128, sbuf_free_dim_per_rank=256,
                    )

            with tc.tile_pool(name="p1", bufs=1) as p1:
                a_idx = p1.tile([128, NA // 16], i16)
                nc.sync.dma_start(a_idx[:], a_idx_in[:])
                gather_chunked(Ag, a_idx, NA)
                n_idx = p1.tile([128, NN // 16], i16)
                nc.sync.dma_start(n_idx[:], n_idx_in[:])
                gather_chunked(NgOwn, n_idx, NN)
                p_idx = p1.tile([128, NP // 16], i16)
                nc.sync.dma_start(p_idx[:], p_idx_in[:])
                Pg = p1.tile([128, 1, NP], bf16)
                gather_chunked(Pg, p_idx, NP)

                # pos partial sum with exact pad correction: pads all point at
                # local row 0, which is also what column NP-1 holds when any
                # pad exists; when there are no pads p_pad is 0.
                p_pad = p1.tile([128, 1], f32)
                nc.sync.dma_start(p_pad[:], p_pad_in[:])
                pos_red = stat.tile([128, 1], f32)
                nc.vector.tensor_reduce(
                    pos_red[:], Pg[:], axis=mybir.AxisListType.XY,
                    op=mybir.AluOpType.add,
                )
                corr = p1.tile([128, 1], f32)
                nc.vector.tensor_scalar_mul(corr[:], Pg[:, 0, NP - 1 : NP], p_pad[:])
                partial = stat.tile([128, 1], f32)
                nc.vector.tensor_sub(partial[:], pos_red[:], corr[:])
                # ship the f32 partial as an exact hi+lo bf16 pair
                p_hilo = stat.tile([128, 2], bf16)
                nc.vector.tensor_copy(p_hilo[:, 0:1], partial[:])
                nc.vector.tensor_sub(p_hilo[:, 1:2], partial[:], p_hilo[:, 0:1])

                # ---- P2: exchange ----
                nc.sync.dma_start(cc_in[:, 0:NN], NgOwn[:, 0, :])
                nc.sync.dma_start(cc_in[:, NN : NN + 2], p_hilo[:])
            nc.gpsimd.collective_compute(
                "AllGather",
                mybir.AluOpType.bypass,
                replica_groups=[list(range(N_CORES))],
                ins=[cc_in[:]],
                outs=[cc_out[:]],
            )

            # compact valid neg columns (drop pads) straight out of DRAM
            NgD = big.tile([128, NN_TOT], bf16)
            off = 0
            for r in range(N_CORES):
                w = nn_list[r]
                if w == 0:
                    continue
                nc.sync.dma_start(
                    NgD[:, off : off + w],
                    cc_out[r, :, 0:w],
                )
                off += w

            # pos_mean from the 8 bitcast partials
            with tc.tile_pool(name="p2", bufs=1) as p2:
                ps8 = p2.tile([128, N_CORES, 2], bf16)
                nc.sync.dma_start(
                    ps8[:],
                    cc_out[:, :, NN : NN + 2].rearrange("r c j -> c r j"),
                )
                pos_sum = p2.tile([128, 1], f32)
                nc.vector.tensor_reduce(
                    pos_sum[:], ps8[:], axis=mybir.AxisListType.XY,
                    op=mybir.AluOpType.add,
                )
                pos_mean = stat.tile([128, 1], bf16)
                nc.scalar.mul(pos_mean[:], pos_sum[:], 1.0 / float(n_pos_total))

            # ---- P3: matmul + exp + rowsum ----
            exp_pos = stat.tile([128, n_atile], f32)
            with tc.tile_pool(name="pps", bufs=1, space="PSUM") as pps:
                zpos = pps.tile([128, n_atile], f32)
                for a in range(n_atile):
                    nc.tensor.matmul(
                        zpos[:, a : a + 1],
                        Ag[:, 0, a * 128 : (a + 1) * 128],
                        pos_mean[:],
                    )
                nc.scalar.activation(
                    exp_pos[:], zpos[:], mybir.ActivationFunctionType.Exp,
                    scale=1.0 / TEMP,
                )

            chunks = []
            o = 0
            while o < NN_TOT:
                w = min(ASSEMBLY_CHUNK, NN_TOT - o)
                chunks.append((o, w))
                o += w

            loss_sb = stat.tile([128, n_atile], f32)
            with (
                tc.tile_pool(name="mmps", bufs=2, space="PSUM") as mmps,
                tc.tile_pool(name="scrap", bufs=3) as scrap_pool,
                tc.tile_pool(name="acc", bufs=2) as acc_pool,
            ):
                for a in range(n_atile):
                    lhsT = Ag[:, 0, a * 128 : (a + 1) * 128]
                    sums = acc_pool.tile([128, len(chunks)], f32)
                    for ci, (o, w) in enumerate(chunks):
                        zp = mmps.tile([128, ASSEMBLY_CHUNK], f32)
                        for s in range(0, w, MM_N):
                            n = min(MM_N, w - s)
                            nc.tensor.matmul(
                                zp[:, s : s + n],
                                lhsT,
                                NgD[:, o + s : o + s + n],
                            )
                        scrap = scrap_pool.tile([128, ASSEMBLY_CHUNK], bf16)
                        nc.scalar.activation(
                            scrap[:, 0:w], zp[:, 0:w],
                            mybir.ActivationFunctionType.Exp,
                            scale=1.0 / TEMP,
                            accum_out=sums[:, ci : ci + 1],
                        )
                    S = acc_pool.tile([128, 1], f32)
                    nc.vector.tensor_reduce(
                        S[:], sums[:], axis=mybir.AxisListType.X,
                        op=mybir.AluOpType.add,
                    )
                    denom = acc_pool.tile([128, 1], f32)
                    nc.vector.tensor_add(denom[:], S[:], exp_pos[:, a : a + 1])
                    rec = acc_pool.tile([128, 1], f32)
                    nc.vector.reciprocal(rec[:], denom[:])
                    nc.vector.tensor_mul(
                        loss_sb[:, a : a + 1], rec[:], exp_pos[:, a : a + 1]
                    )

            nc.sync.dma_start(loss_out[:], loss_sb[:])

    nc.finalize()
    return nc




def _route(anchor_idx, pos_idx, neg_idx):
    """Route index entries to the owning core as local indices."""
    anchor_idx = np.asarray(anchor_idx).astype(np.int64)
    pos_idx = np.asarray(pos_idx).astype(np.int64)
    neg_idx = np.asarray(neg_idx).astype(np.int64)

    a_owner, a_local = anchor_idx // RANGE, anchor_idx % RANGE
    p_owner, p_local = pos_idx // RANGE, pos_idx % RANGE
    n_owner, n_local = neg_idx // RANGE, neg_idx % RANGE

    a_lists, p_lists, n_lists, perms = [], [], [], []
    for k in range(N_CORES):
        sel = np.nonzero(a_owner == k)[0]
        perms.append(sel)
        a_lists.append(a_local[sel])
        p_lists.append(p_local[p_owner == k])
        n_lists.append(n_local[n_owner == k])

    NA = max(128, _ceil_to(max(len(x) for x in a_lists), 128))
    NP = max(128, _ceil_to(max(len(x) for x in p_lists), 128))
    NN = max(128, _ceil_to(max(len(x) for x in n_lists), 128))
    nn_list = [len(x) for x in n_lists]
    return a_lists, p_lists, n_lists, perms, NA, NP, NN, nn_list


def make_in_maps(fslices, a_lists, p_lists, n_lists, NA, NP, NN):
    in_maps = []
    for k in range(N_CORES):
        in_maps.append(
            {
                "feat": fslices[k],
                "a_idx": _wrap_idx16(a_lists[k], NA),
                "n_idx": _wrap_idx16(n_lists[k], NN),
                "p_idx": _wrap_idx16(p_lists[k], NP),
                "p_pad": np.full((128, 1), NP - len(p_lists[k]), dtype=np.float32),
            }
        )
    return in_maps


def _prepare(feat, anchor_idx, pos_idx, neg_idx):
    """Host-side sharding: slice feat per core, route index entries to the
    owning core as local int16 indices, pad to common shapes."""
    feat = np.ascontiguousarray(np.asarray(feat), dtype=np.float32)
    fview = feat.reshape(B, C, (H * W))

    a_lists, p_lists, n_lists, perms, NA, NP, NN, nn_list = _route(
        anchor_idx, pos_idx, neg_idx
    )

    per_image = H * W // RANGE  # core ranges per image
    fslices = []
    for k in range(N_CORES):
        b, part = k // per_image, k % per_image
        fslices.append(
            np.ascontiguousarray(fview[b, :, part * RANGE : (part + 1) * RANGE])
        )
    in_maps = make_in_maps(fslices, a_lists, p_lists, n_lists, NA, NP, NN)
    return in_maps, perms, a_lists, NA, NP, NN, nn_list


def run_sharded(feat, anchor_idx, pos_idx, neg_idx, trace=False):
    from concourse.bass_utils import run_bass_kernel_spmd

    in_maps, perms, a_lists, NA, NP, NN, nn_list = _prepare(
        feat, anchor_idx, pos_idx, neg_idx
    )
    nc = _build_program(NA, NN, NP, nn_list, int(np.asarray(pos_idx).size))
    res = run_bass_kernel_spmd(
        nc, in_maps, core_ids=list(range(N_CORES)), trace=trace
    )
    n_anchor = sum(len(p) for p in perms)
    loss = np.zeros(n_anchor, dtype=np.float32)
    for k in range(N_CORES):
        buf = res.results[k]["loss"]  # [128, NA//128]
        vals = buf.T.reshape(-1)[: len(perms[k])]
        loss[perms[k]] = vals
    return loss, res


def kernel(feat, anchor_idx, pos_idx, neg_idx):
    out_dtype = np.asarray(feat).dtype
    loss, _ = run_sharded(feat, anchor_idx, pos_idx, neg_idx, trace=False)
    return loss.astype(out_dtype)
